# revision 1
# baseline (speedup 1.0000x reference)
"""ARERec forward kernel for 8 TRN2 NeuronCores.

Data-parallel over batch: each core processes B/8 = 64 batch rows end-to-end
(embedding gathers, single-query multi-head attention, LCU region profiles,
rating classifier); tables and weights are replicated. The final [512, 5]
softmax output is concatenated on the host from the 8 per-core [64, 5] shards.

Self-contained: shapes/sharding are hardcoded from the problem spec.
"""
import numpy as np
import ml_dtypes

import concourse.bacc as bacc
import concourse.bass as bass
import concourse.mybir as mybir
import concourse.tile as tile
from concourse.masks import make_identity
from concourse.bass_utils import run_bass_kernel_spmd

NCORES = 8
B, S, E, H, R = 512, 50, 128, 8, 32
D = E // H
USERS, ITEMS, ALLSEQ, NCLASS = 50000, 20000, 20000, 5
BC = B // NCORES            # 64 batch rows per core
J = BC * S                  # 3200 (b, s) pairs per core
G = J // 128                # 25 gather groups of 128 pairs
# b-aligned chunks (multiples of S); two small priming chunks let the
# attention->profile software pipeline fill faster
CH400 = [(0, 200), (200, 200)] + [(c * 400, 400) for c in range(1, J // 400)]
NFOLD = 7                   # fold all the way to width 1
NDMAFOLD = 8                # groups 13,15,17,19,21..24: first fold on DMA engines

F32 = mybir.dt.float32
BF16 = mybir.dt.bfloat16
I32 = mybir.dt.int32
AF = mybir.ActivationFunctionType
ALU = mybir.AluOpType


def _ap(ap, dims):
    """Rebuild an AP with explicit [step, count] free dims (partition dim kept)."""
    return bass.AP(tensor=ap.tensor, offset=ap.offset, ap=[ap.ap[0]] + dims)


def build_nc(repeat=1):
    nc = bacc.Bacc(None, target_bir_lowering=False)

    p_user = nc.declare_dram_parameter("user_i", [BC, 1], I32, isOutput=False)
    p_nbg = nc.declare_dram_parameter("nbg_i", [128, G], I32, isOutput=False)
    p_seq = nc.declare_dram_parameter("seq_i", [128, G], I32, isOutput=False)
    p_itg = nc.declare_dram_parameter("itg_i", [128, G], I32, isOutput=False)
    p_negm = nc.declare_dram_parameter("negmask", [1, J], F32, isOutput=False)
    p_uemb = nc.declare_dram_parameter("uemb", [USERS, E], F32, isOutput=False)
    p_item = nc.declare_dram_parameter("item_bf", [ITEMS, E], BF16, isOutput=False)
    p_kiu = nc.declare_dram_parameter("lcu_iu", [ALLSEQ, R * E], BF16, isOutput=False)
    p_kui = nc.declare_dram_parameter("lcu_ui", [ALLSEQ, R * E], BF16, isOutput=False)
    p_wq = nc.declare_dram_parameter("wq", [E, E], F32, isOutput=False)
    p_wk = nc.declare_dram_parameter("wk", [E, E], F32, isOutput=False)
    p_wv = nc.declare_dram_parameter("wv", [E, E], F32, isOutput=False)
    p_wo = nc.declare_dram_parameter("wo", [E, E], F32, isOutput=False)
    p_bias = nc.declare_dram_parameter("biases", [E, 4], F32, isOutput=False)
    p_sel_eh = nc.declare_dram_parameter("sel_eh", [E, H], F32, isOutput=False)
    p_sel_he = nc.declare_dram_parameter("sel_he", [H, E], F32, isOutput=False)
    p_fcwb = nc.declare_dram_parameter("fc_wb", [R + 1, NCLASS], F32, isOutput=False)
    p_out = nc.declare_dram_parameter("out", [BC, NCLASS], F32, isOutput=True)
    p_ur = nc.declare_dram_parameter("ur_dbg", [R, BC], F32, isOutput=True)

    with tile.TileContext(nc) as tc:
        with (
            tc.tile_pool(name="const", bufs=1) as cpool,
            tc.tile_pool(name="big", bufs=1) as bpool,
            tc.tile_pool(name="work", bufs=3) as wpool,
            tc.tile_pool(name="kwork", bufs=4) as kpool,
            tc.tile_pool(name="ps_big", bufs=4, space="PSUM") as pp_big,
            tc.tile_pool(name="ps_lg", bufs=2, space="PSUM") as pp_lg,
            tc.tile_pool(name="ps_rt", bufs=1, space="PSUM") as pp_rt,
            tc.tile_pool(name="ps_l2", bufs=1, space="PSUM") as pp_l2,
        ):
            # ---------- constants (loaded once, outside the repeat loop) ----------
            t_ident = cpool.tile([128, 128], F32)
            make_identity(nc, t_ident[:])
            t_user = cpool.tile([BC, 1], I32)
            nc.sync.dma_start(out=t_user[:], in_=p_user[:])
            t_nbg = cpool.tile([128, G], I32)
            nc.sync.dma_start(out=t_nbg[:], in_=p_nbg[:])
            t_seq = cpool.tile([128, G], I32)
            nc.sync.dma_start(out=t_seq[:], in_=p_seq[:])
            t_itg = cpool.tile([128, G], I32)
            nc.sync.dma_start(out=t_itg[:], in_=p_itg[:])
            t_negm = cpool.tile([1, J], F32)
            nc.sync.dma_start(out=t_negm[:], in_=p_negm[:])
            t_wq = cpool.tile([E, E], F32)
            nc.sync.dma_start(out=t_wq[:], in_=p_wq[:])
            t_wk = cpool.tile([E, E], F32)
            nc.sync.dma_start(out=t_wk[:], in_=p_wk[:])
            t_wv = cpool.tile([E, E], F32)
            nc.sync.dma_start(out=t_wv[:], in_=p_wv[:])
            t_wo = cpool.tile([E, E], F32)
            nc.sync.dma_start(out=t_wo[:], in_=p_wo[:])
            t_bias = cpool.tile([E, 4], F32)
            nc.sync.dma_start(out=t_bias[:], in_=p_bias[:])
            t_sel_eh = cpool.tile([E, H], F32)
            nc.sync.dma_start(out=t_sel_eh[:], in_=p_sel_eh[:])
            t_sel_he = cpool.tile([H, E], F32)
            nc.sync.dma_start(out=t_sel_he[:], in_=p_sel_he[:])
            t_fcwb = cpool.tile([R + 1, NCLASS], F32)
            nc.sync.dma_start(out=t_fcwb[:], in_=p_fcwb[:])
            t_ones18 = cpool.tile([1, H], F32)
            nc.vector.memset(t_ones18[:], 1.0)
            # per-pair padding mask (1.0 where neighbor > 0), [128, G]
            t_wcol = cpool.tile([128, G], F32)
            nc.vector.tensor_scalar(out=t_wcol[:], in0=t_nbg[:], scalar1=0,
                                    scalar2=None, op0=ALU.is_gt)

            def body():
                # user rows -> [BC, E] -> transpose -> uT [E, BC]
                t_u = wpool.tile([BC, E], F32, tag="gath_u")
                nc.gpsimd.indirect_dma_start(
                    out=t_u[:], out_offset=None, in_=p_uemb[:],
                    in_offset=bass.IndirectOffsetOnAxis(ap=t_user[:, :1], axis=0))
                ps_uT = pp_big.tile([E, BC], F32, tag="big")
                nc.tensor.transpose(out=ps_uT[:], in_=t_u[:], identity=t_ident[:BC, :BC])
                t_uT = bpool.tile([E, BC], F32)
                nc.scalar.copy(out=t_uT[:], in_=ps_uT[:])

                # qT = (wq.T @ uT + bq) * (1/sqrt(D))
                ps_q = pp_big.tile([E, BC], F32, tag="big")
                nc.tensor.matmul(out=ps_q[:], lhsT=t_wq[:], rhs=t_uT[:])
                t_qT = bpool.tile([E, BC], F32)
                nc.scalar.activation(out=t_qT[:], in_=ps_q[:], func=AF.Identity,
                                     bias=t_bias[:, 0:1], scale=1.0)

                t_nbT = bpool.tile([E, J], F32)
                t_kT = bpool.tile([E, J], F32)
                t_vT = bpool.tile([E, J], F32)
                t_att = bpool.tile([H, J], F32)
                t_oT = bpool.tile([E, J], F32)
                t_ratT = bpool.tile([R, J], F32)

                def emit_nb_group(g):
                    # neighbor rows for group g, transposed into nbT slice
                    t_nb = wpool.tile([128, E], F32, tag="gath_nb")
                    nc.gpsimd.indirect_dma_start(
                        out=t_nb[:], out_offset=None, in_=p_uemb[:],
                        in_offset=bass.IndirectOffsetOnAxis(ap=t_nbg[:, g:g + 1], axis=0))
                    ps_t = pp_big.tile([128, 128], F32, tag="big")
                    nc.tensor.transpose(out=ps_t[:], in_=t_nb[:], identity=t_ident[:])
                    nc.scalar.copy(out=t_nbT[:, g * 128:(g + 1) * 128], in_=ps_t[:])

                def emit_attn_a(ci, c0, cn):
                    sl = slice(c0, c0 + cn)
                    nb = cn // S
                    bsl = slice(c0 // S, (c0 + cn) // S)
                    # k/v projections for this chunk
                    ps_k = pp_big.tile([E, 400], F32, tag="big")
                    nc.tensor.matmul(out=ps_k[:, :cn], lhsT=t_wk[:], rhs=t_nbT[:, sl])
                    nc.scalar.activation(out=t_kT[:, sl], in_=ps_k[:, :cn],
                                         func=AF.Identity, bias=t_bias[:, 1:2], scale=1.0)
                    ps_v = pp_big.tile([E, 400], F32, tag="big")
                    nc.tensor.matmul(out=ps_v[:, :cn], lhsT=t_wv[:], rhs=t_nbT[:, sl])
                    nc.scalar.activation(out=t_vT[:, sl], in_=ps_v[:, :cn],
                                         func=AF.Identity, bias=t_bias[:, 2:3], scale=1.0)
                    # prod_qk = kT * qT / sqrt(D) (in place over kT)
                    kT3 = t_kT[:, sl].rearrange("e (b s) -> e b s", s=S)
                    nc.vector.scalar_tensor_tensor(
                        out=kT3, in0=kT3, scalar=1.0 / np.sqrt(D),
                        in1=_ap(t_qT[:, bsl], [[1, nb], [0, S]]),
                        op0=ALU.mult, op1=ALU.mult)
                    # logits = per-head sums + negmask; attn = exp(logits)
                    # (masked entries underflow to exactly 0, matching the
                    # reference softmax); normalized by the row sum below.
                    ps_lg = pp_lg.tile([H, 400], F32, tag="lg")
                    nc.tensor.matmul(out=ps_lg[:, :cn], lhsT=t_sel_eh[:], rhs=t_kT[:, sl],
                                     start=True, stop=False)
                    nc.tensor.matmul(out=ps_lg[:, :cn], lhsT=t_ones18[:], rhs=t_negm[:, sl],
                                     start=False, stop=True)
                    t_ssc = wpool.tile([H, 8], F32, tag="ssc")
                    ss_tiles[ci] = t_ssc
                    for bi in range(cn // S):
                        nc.scalar.activation(out=t_att[:, c0 + bi * S:c0 + (bi + 1) * S],
                                             in_=ps_lg[:, bi * S:(bi + 1) * S], func=AF.Exp,
                                             accum_out=t_ssc[:, bi:bi + 1])

                def emit_attn_b(ci, c0, cn):
                    sl = slice(c0, c0 + cn)
                    nb = cn // S
                    att3 = t_att[:, sl].rearrange("h (b s) -> h b s", s=S)
                    t_ssc = ss_tiles.pop(ci)
                    t_rs = wpool.tile([H, 8], F32, tag="sm2")
                    nc.vector.reciprocal(out=t_rs[:, :nb], in_=t_ssc[:, :nb])
                    nc.vector.tensor_tensor(out=att3, in0=att3,
                                            in1=_ap(t_rs[:, :nb], [[1, nb], [0, S]]),
                                            op=ALU.mult)
                    # ctxT = attn_bcast * vT (in place over vT); ctxo = wo.T@ctx + bo
                    ps_ab = pp_big.tile([E, 400], F32, tag="big")
                    nc.tensor.matmul(out=ps_ab[:, :cn], lhsT=t_sel_he[:], rhs=t_att[:, sl])
                    nc.vector.tensor_tensor(out=t_vT[:, sl], in0=t_vT[:, sl],
                                            in1=ps_ab[:, :cn], op=ALU.mult)
                    ps_o = pp_big.tile([E, 400], F32, tag="big")
                    nc.tensor.matmul(out=ps_o[:, :cn], lhsT=t_wo[:], rhs=t_vT[:, sl])
                    nc.scalar.activation(out=t_oT[:, sl], in_=ps_o[:, :cn],
                                         func=AF.Identity, bias=t_bias[:, 3:4], scale=1.0)

                mul_tiles = {}
                k_tiles = {}
                ss_tiles = {}

                def emit_k_gather(g):
                    # both LCU tables' rows stacked into one [128, 2*R*E] tile
                    t_k = kpool.tile([128, 2 * R * E], BF16, tag="k")
                    k_tiles[g] = t_k
                    nc.gpsimd.indirect_dma_start(
                        out=t_k[:, 0:R * E], out_offset=None, in_=p_kiu[:],
                        in_offset=bass.IndirectOffsetOnAxis(ap=t_seq[:, g:g + 1], axis=0))
                    nc.gpsimd.indirect_dma_start(
                        out=t_k[:, R * E:2 * R * E], out_offset=None, in_=p_kui[:],
                        in_offset=bass.IndirectOffsetOnAxis(ap=t_seq[:, g:g + 1], axis=0))

                def emit_profile_prep(g):
                    # multiplier stack: [128, 2*E] = (ctxo pair-major | item rows)
                    t_mul = wpool.tile([128, 2 * E], BF16, tag="mul")
                    mul_tiles[g] = t_mul
                    ps_tp = pp_big.tile([128, 128], F32, tag="big")
                    nc.tensor.transpose(out=ps_tp[:], in_=t_oT[:, g * 128:(g + 1) * 128],
                                        identity=t_ident[:])
                    nc.scalar.copy(out=t_mul[:, 0:E], in_=ps_tp[:])
                    nc.gpsimd.indirect_dma_start(
                        out=t_mul[:, E:2 * E], out_offset=None, in_=p_item[:],
                        in_offset=bass.IndirectOffsetOnAxis(ap=t_itg[:, g:g + 1], axis=0))

                def emit_profile_compute(g):
                    t_mul = mul_tiles.pop(g)
                    t_k = k_tiles.pop(g)
                    # prod = K * multiplier; LCU rows are host-permuted to
                    # (e_msb, r, e_low) so the first fold halves are contiguous
                    k5 = t_k[:].rearrange("p (t m r e) -> p t m r e", t=2, m=2, r=R)
                    mul_b = _ap(t_mul[:], [[E, 2], [E // 2, 2], [0, R], [1, E // 2]])
                    nc.vector.tensor_tensor(out=k5, in0=k5, in1=mul_b, op=ALU.mult)
                    # fold over e_msb: contiguous 2048-elem halves per table --
                    # on the DMA engines (CCE accum-add) for the first groups,
                    # on DVE for the rest (load balance)
                    if g >= G - 12 and (g % 2 == 1 or g >= G - 4):
                        nc.gpsimd.dma_start(
                            out=_ap(t_k[:], [[2 * R * E // 2, 2], [1, R * E // 2]]),
                            in_=_ap(t_k[:], [[2 * R * E // 2, 2], [1, R * E // 2]],
                                    ) if False else bass.AP(
                                        tensor=t_k[:].tensor,
                                        offset=t_k[:].offset + R * E // 2,
                                        ap=[t_k[:].ap[0], [2 * R * E // 2, 2], [1, R * E // 2]]),
                            accum_op=ALU.add)
                    else:
                        nc.vector.tensor_tensor(out=k5[:, :, 0, :, :],
                                                in0=k5[:, :, 0, :, :],
                                                in1=k5[:, :, 1, :, :], op=ALU.add)
                    k4 = t_k[:].rearrange("p (t m r e) -> p t m r e",
                                          t=2, m=2, r=R)[:, :, 0, :, :]
                    w = E // 2
                    for _ in range(NFOLD - 1):
                        h = w // 2
                        nc.vector.tensor_tensor(out=k4[:, :, :, 0:h],
                                                in0=k4[:, :, :, 0:h],
                                                in1=k4[:, :, :, h:w], op=ALU.add)
                        w = h
                    # rating = nprof * w * iprof straight from the folded
                    # (width-1) bf16 sums, read with stride E over r
                    t_rat = wpool.tile([128, R], F32, tag="rat")
                    nc.vector.scalar_tensor_tensor(out=t_rat[:],
                                                   in0=k4[:, 0, :, 0:1].rearrange("p r o -> p (r o)"),
                                                   scalar=t_wcol[:, g:g + 1],
                                                   in1=k4[:, 1, :, 0:1].rearrange("p r o -> p (r o)"),
                                                   op0=ALU.mult, op1=ALU.mult)
                    ps_rt = pp_rt.tile([R, 128], F32, tag="rt")
                    nc.tensor.transpose(out=ps_rt[:], in_=t_rat[:], identity=t_ident[:])
                    nc.scalar.copy(out=t_ratT[:, g * 128:(g + 1) * 128], in_=ps_rt[:])

                # software-pipelined emission: neighbor gathers -> attention
                # stage A -> profile preps (gathers/multipliers run ahead) ->
                # profile computes (fill DVE) -> attention stage B
                g_nb = 0
                g_kg = 0
                g_prep = 0
                g_comp = 0
                for ci, (c0, cn) in enumerate(CH400):
                    hi = c0 + cn
                    while g_nb * 128 < hi:
                        emit_nb_group(g_nb)
                        g_nb += 1
                    while (g_kg + 1) * 128 <= hi:
                        emit_k_gather(g_kg)
                        g_kg += 1
                    emit_attn_a(ci, c0, cn)
                    while g_comp < g_prep - 1:
                        emit_profile_compute(g_comp)
                        g_comp += 1
                    emit_attn_b(ci, c0, cn)
                    while (g_prep + 1) * 128 <= hi:
                        emit_profile_prep(g_prep)
                        g_prep += 1
                while g_kg < G:
                    emit_k_gather(g_kg)
                    g_kg += 1
                while g_prep < G:
                    emit_profile_prep(g_prep)
                    g_prep += 1
                while g_comp < G:
                    emit_profile_compute(g_comp)
                    g_comp += 1

                # user rating vector: max over s
                t_urp = wpool.tile([R + 1, BC], F32, tag="urp")
                nc.vector.tensor_reduce(out=t_urp[:R, :],
                                        in_=t_ratT[:].rearrange("r (b s) -> r b s", s=S),
                                        axis=mybir.AxisListType.X, op=ALU.max)
                nc.vector.memset(t_urp[R:R + 1, :], 1.0)
                nc.sync.dma_start(out=p_ur[:], in_=t_urp[:R, :])

                # classifier + softmax
                ps_l2 = pp_l2.tile([BC, NCLASS], F32, tag="l2")
                nc.tensor.matmul(out=ps_l2[:], lhsT=t_urp[:], rhs=t_fcwb[:])
                t_nm2 = wpool.tile([BC, 1], F32, tag="fin")
                nc.vector.tensor_reduce(out=t_nm2[:], in_=ps_l2[:],
                                        axis=mybir.AxisListType.X,
                                        op=ALU.max, negate=True)
                t_e2 = wpool.tile([BC, NCLASS], F32, tag="fin2")
                t_s2 = wpool.tile([BC, 1], F32, tag="fin3")
                nc.scalar.activation(out=t_e2[:], in_=ps_l2[:], func=AF.Exp,
                                     bias=t_nm2[:, :1], scale=1.0,
                                     accum_out=t_s2[:, :1])
                t_r2 = wpool.tile([BC, 1], F32, tag="fin4")
                nc.vector.reciprocal(out=t_r2[:], in_=t_s2[:])
                t_o = wpool.tile([BC, NCLASS], F32, tag="fin5")
                nc.vector.tensor_scalar(out=t_o[:], in0=t_e2[:], scalar1=t_r2[:, :1],
                                        scalar2=None, op0=ALU.mult)
                nc.sync.dma_start(out=p_out[:], in_=t_o[:])

            if repeat == 1:
                body()
            else:
                with tc.For_i(0, repeat, 1):
                    body()

    nc.finalize()
    return nc


def prep_in_maps(inputs):
    user = np.asarray(inputs["user"]).astype(np.int32).reshape(B)
    item = np.asarray(inputs["item"]).astype(np.int32).reshape(B)
    neighbor = np.asarray(inputs["neighbor"]).astype(np.int32).reshape(B, S)
    seq = np.asarray(inputs["seq"]).astype(np.int32).reshape(B, S)

    f32 = lambda x: np.ascontiguousarray(np.asarray(x, dtype=np.float32))
    bf16 = lambda x: np.ascontiguousarray(
        np.asarray(x, dtype=np.float32).astype(ml_dtypes.bfloat16))

    uemb = f32(inputs["user_emb_table"])
    item_bf = bf16(inputs["item_emb_table"])
    perm = lambda t: np.ascontiguousarray(
        t.reshape(ALLSEQ, R, 2, E // 2).transpose(0, 2, 1, 3).reshape(ALLSEQ, R * E))
    lcu_iu = perm(bf16(inputs["item_user_LCU"]))
    lcu_ui = perm(bf16(inputs["user_item_LCU"]))
    biases = np.ascontiguousarray(np.stack(
        [f32(inputs["bq"]), f32(inputs["bk"]),
         f32(inputs["bv"]), f32(inputs["bo"])], axis=1))
    sel_eh = np.zeros((E, H), np.float32)
    sel_eh[np.arange(E), np.arange(E) // D] = 1.0
    fc_wb = np.ascontiguousarray(np.concatenate(
        [f32(inputs["fc_w"]), f32(inputs["fc_b"]).reshape(1, NCLASS)], axis=0))

    shared = {
        "uemb": uemb, "item_bf": item_bf, "lcu_iu": lcu_iu, "lcu_ui": lcu_ui,
        "wq": f32(inputs["wq"]), "wk": f32(inputs["wk"]),
        "wv": f32(inputs["wv"]), "wo": f32(inputs["wo"]),
        "biases": biases, "sel_eh": np.ascontiguousarray(sel_eh),
        "sel_he": np.ascontiguousarray(sel_eh.T), "fc_wb": fc_wb,
    }
    in_maps = []
    for c in range(NCORES):
        bsl = slice(c * BC, (c + 1) * BC)
        nb = neighbor[bsl].reshape(J)
        sq = seq[bsl].reshape(J)
        itx = np.repeat(item[bsl], S)
        col = lambda x: np.ascontiguousarray(x.reshape(G, 128).T.astype(np.int32))
        in_maps.append({
            **shared,
            "user_i": np.ascontiguousarray(user[bsl].reshape(BC, 1)),
            "nbg_i": col(nb), "seq_i": col(sq), "itg_i": col(itx),
            "negmask": np.ascontiguousarray(
                (-1e9 * (nb <= 0)).astype(np.float32).reshape(1, J)),
        })
    return in_maps


_NC_CACHE = {}


def kernel(**inputs):
    if "nc" not in _NC_CACHE:
        _NC_CACHE["nc"] = build_nc(repeat=1)
    nc = _NC_CACHE["nc"]
    in_maps = prep_in_maps(inputs)
    res = run_bass_kernel_spmd(nc, in_maps, core_ids=list(range(NCORES)))
    return np.concatenate([res.results[c]["out"] for c in range(NCORES)], axis=0)

